# revision 14
# baseline (speedup 1.0000x reference)
"""ViT-style attention with decomposed relative position embeddings on 8 TRN2
NeuronCores. Data-parallel over batch (B=8 -> 1 image per core); weights and
the small rel-pos tables are replicated.

The kernel is ScalarE-bound (96 exp instructions ~1.15us each), so the
schedule (a) starts the exp stream as early as possible and (b) fills the PE
inside the exp-rate-limited attention phase with deferred matmuls (v-GEMM,
k o-tiles, proj, rel_w transposes) through spare PSUM bank slots.

Scheduling laws learned from traces:
  - engines execute their queues in FIFO emission order, so ScalarE may only
    carry pre-exp work that retires early (q/k/v evacuations, gated on the
    early o-tiles); anything gated on the late rel matmuls goes to VectorE.
  - matmul rhs APs must be contiguous (stride-12 rhs = 2.8x slower), so
    qext/kext use the head-major [128, 12h, 32a, 32b] layout.
  - scattered DVE/gpsimd copies cost ~3x their isolated time under load, so
    the rel_w a<->b transpose runs on the PE as an identity matmul with a
    scatter-streamed rhs (output lands contiguous in PSUM).

Math folds that remove whole pipelines:
  - the key-projection bias only adds a per-query constant to the logits,
    which softmax cancels -> dropped entirely.
  - the value bias passes through softmax unchanged (weights sum to 1) ->
    folded host-side into the proj bias.
  - the query bias rides the qk GEMM as a K=1 ones-row matmul, so all
    projection evacuations are plain copies.

Per-core computation (one image, T=1024 tokens, C=768, 12 heads x 64):
  - S^T = Kext^T . Qext per head and query-half (hf: 512 q's); contraction
    128 = 64 qk dims + 32 rel_h rows + 32 rel_w rows vs onehot rows of Kext,
    so the decomposed rel-pos additions ride in the S matmul for free.
  - exp on ScalarE in [128, 2, 512] chunks -> P^T (bf16)
  - PV: out^T[65, 512] = Vaug^T . P^T accumulated over 8 k-chunks; ones
    column of Vaug makes row 64 the softmax denominator.
  - normalization: denominator row DVE-copied + DMA-reshaped [1,512]->
    [32,16], DVE reciprocal, DMA back + gpsimd partition-broadcast, one DVE
    multiply (all off the Scalar/Tensor critical path).
  - attention in 24 slots = (hf=0: h0..h11, hf=1: h0..h11); PV of slot s
    issues during slot s+2 so the exp stream never waits on PSUM.
  - proj for query-half 0 runs as filler during half-1 slots; half-1 proj
    tiles accumulate progressively while the last PV/norm chains drain.

All matmuls bf16 (fp32 PSUM accumulation).
"""

import contextlib

import numpy as np
import ml_dtypes

BF16 = ml_dtypes.bfloat16

B, H, W, C = 8, 32, 32, 768
NH, HD, T = 12, 64, 1024
N_CORES = 8

_cache = {}


def _bf(a):
    return np.ascontiguousarray(np.asarray(a, dtype=np.float32)).astype(BF16)


def _f32(a):
    return np.ascontiguousarray(np.asarray(a, dtype=np.float32))


def _build_nc():
    if "nc" in _cache:
        return _cache["nc"]

    import concourse.mybir as mybir
    import concourse.tile as tile
    from concourse import bacc

    f32 = mybir.dt.float32
    bf16 = mybir.dt.bfloat16
    EXP = mybir.ActivationFunctionType.Exp

    nc = bacc.Bacc("TRN2", target_bir_lowering=False, debug=False)

    # ---- DRAM I/O ----
    xT_d = nc.dram_tensor("xT", [C, T], bf16, kind="ExternalInput")
    wqk_d = nc.dram_tensor("w_qk", [C, 2 * C], bf16, kind="ExternalInput")
    wv_d = nc.dram_tensor("w_v", [C, C], bf16, kind="ExternalInput")
    wp_d = nc.dram_tensor("w_p", [C, C], bf16, kind="ExternalInput")
    bqk_d = nc.dram_tensor("b_qk", [1, C], bf16, kind="ExternalInput")
    bp_d = nc.dram_tensor("b_p", [1, C], f32, kind="ExternalInput")
    relt_d = nc.dram_tensor("relt", [64, 2048], bf16, kind="ExternalInput")
    oneh_d = nc.dram_tensor("onehot", [64, 32, 32], bf16, kind="ExternalInput")
    iden_d = nc.dram_tensor("ident", [32, 32], bf16, kind="ExternalInput")
    out_d = nc.dram_tensor("out", [T, C], f32, kind="ExternalOutput")

    with tile.TileContext(nc) as tc:
        es = contextlib.ExitStack()
        cp = es.enter_context(tc.tile_pool(name="const", bufs=1))

        # ---- persistent SBUF tensors ----
        xT = cp.tile([128, 6, T], bf16, tag="xT")
        wqk = cp.tile([128, 6, 2 * C], bf16, tag="wqk")
        wv = cp.tile([128, 6, C], bf16, tag="wv")
        wpr = cp.tile([128, 6, C], bf16, tag="wpr")
        bqk = cp.tile([1, C], bf16, tag="bqk")
        ones_row = cp.tile([1, T], bf16, tag="ones_row")
        bp_row = cp.tile([1, C], f32, tag="bp_row")
        bp_bc = cp.tile([128, C], f32, tag="bp_bc")
        relt = cp.tile([64, 2048], bf16, tag="relt")
        iden = cp.tile([32, 32], bf16, tag="iden")
        # qext: [part, head, a(row), b(col)]; rows 0:64 = q/8 (+bias),
        # rows 64:96 rel_h (kh j), rows 96:128 rel_w (kw j)
        qext = cp.tile([128, NH, 32, 32], bf16, tag="qext")
        # kext: [part, head, a, b]; rows 0:64 = k, 64:128 onehot
        kext = cp.tile([128, NH, 32, 32], bf16, tag="kext")
        vaug = cp.tile([128, 8, NH, 65], bf16, tag="vaug")
        yall = cp.tile([128, 6, T], bf16, tag="yall")
        # rel_w staging: [j, b, head, a] (per-b slabs land contiguous)
        stg_w = cp.tile([32, 32, NH, 32], bf16, tag="stg_w")

        # P buffers (exp output) - 3 bufs to support PV deferral of 2 slots
        pp = es.enter_context(tc.tile_pool(name="pbuf", bufs=3))
        np_pool = es.enter_context(tc.tile_pool(name="norm", bufs=2))
        zp = es.enter_context(tc.tile_pool(name="zout", bufs=2))

        # ---- input DMAs (interleaved so matmul (ot=0, c=0) starts early) ----
        nc.sync.dma_start(bqk[:], bqk_d[:])
        for c in range(6):
            nc.sync.dma_start(xT[:, c, :], xT_d[c * 128:(c + 1) * 128, :])
            nc.sync.dma_start(wqk[:, c, :], wqk_d[c * 128:(c + 1) * 128, :])
        for c in range(6):
            nc.sync.dma_start(wv[:, c, :], wv_d[c * 128:(c + 1) * 128, :])
        nc.sync.dma_start(relt[:], relt_d[:])
        nc.sync.dma_start(iden[:], iden_d[:])
        for h in range(NH):
            nc.sync.dma_start(kext[64:128, h, :, :], oneh_d[:])
        nc.sync.dma_start(bp_row[:], bp_d[:])
        for c in range(6):
            nc.sync.dma_start(wpr[:, c, :], wp_d[c * 128:(c + 1) * 128, :])
        nc.gpsimd.partition_broadcast(bp_bc[:], bp_row[:])
        nc.gpsimd.memset(vaug[:, :, :, 64:65], 1.0)
        nc.gpsimd.memset(ones_row[:], 1.0)

        # ======== phase A: q o-tiles, then k pairs 0-1 ========
        def qk_otile(ps, ot, tag="qk"):
            # q bias rides as a K=1 ones-row matmul so the evacuation is a
            # plain copy; phase-A copies go to ScalarE (they retire early and
            # must not delay the exp stream), filler copies to VectorE.
            is_q = ot < 6
            acc = ps.tile([128, 32, 32], f32, tag=tag, name=f"qk_{ot}")
            for c in range(6):
                for hf in range(2):
                    nc.tensor.matmul(
                        acc[:, hf * 16:(hf + 1) * 16, :],
                        wqk[:, c, ot * 128:(ot + 1) * 128],
                        xT[:, c, hf * 512:(hf + 1) * 512],
                        start=(c == 0), stop=(c == 5) and not is_q,
                    )
            if is_q:
                for hf in range(2):
                    nc.tensor.matmul(
                        acc[:, hf * 16:(hf + 1) * 16, :],
                        bqk[0:1, ot * 128:(ot + 1) * 128],
                        ones_row[0:1, hf * 512:(hf + 1) * 512],
                        start=False, stop=True,
                    )
            hp = ot if is_q else ot - 6
            for half in range(2):
                head = 2 * hp + half
                src = acc[64 * half:64 * (half + 1), :, :]
                dst = (qext if is_q else kext)[0:64, head, :, :]
                if tag == "aux":
                    nc.vector.tensor_copy(dst, src)
                else:
                    nc.scalar.copy(dst, src)

        with tc.tile_pool(name="ps_qk", bufs=2, space="PSUM") as ps_qk:
            for ot in (0, 1, 2, 3, 4, 5, 6, 7):
                qk_otile(ps_qk, ot)

        # pools (released LIFO; ps_rel released mid-kernel -> banks to ps_pv)
        ps_aux = es.enter_context(
            tc.tile_pool(name="ps_aux", bufs=1, space="PSUM"))
        ps_s = es.enter_context(
            tc.tile_pool(name="ps_s", bufs=2, space="PSUM"))
        rel_es = contextlib.ExitStack()
        ps_rel = rel_es.enter_context(
            tc.tile_pool(name="ps_rel", bufs=2, space="PSUM"))

        def v_tile(tt):
            accv = ps_aux.tile([128, 32, 32], f32, tag="aux",
                               name=f"v_{tt}").rearrange(
                                   "p a b -> p (a b)")[:, 0:768].rearrange(
                                   "p (h d) -> p h d", h=NH)
            for c in range(6):
                nc.tensor.matmul(
                    accv[:, 0:8, :],
                    xT[:, c, tt * 128:(tt + 1) * 128],
                    wv[:, c, 0:512],
                    start=(c == 0), stop=(c == 5),
                )
                nc.tensor.matmul(
                    accv[:, 8:12, :],
                    xT[:, c, tt * 128:(tt + 1) * 128],
                    wv[:, c, 512:768],
                    start=(c == 0), stop=(c == 5),
                )
            if tt < 6:
                nc.scalar.copy(vaug[:, tt, :, 0:64], accv[:])
            else:
                nc.vector.tensor_copy(vaug[:, tt, :, 0:64], accv[:])

        # ---- rel block: 32 iterations (one image row/col x).
        # rel_h (x=a): out [j, (h, b)] partitions 64:96, evac direct.
        # rel_w (x=b): out [j, (h, a)] partitions 96:128, evac to staging;
        # per-head PE identity-matmuls transpose staging -> qext rows 96:128.
        def rel_iter(x):
            accr = ps_rel.tile([128, NH, 32], f32, tag="rel")
            nc.tensor.matmul(
                accr[64:96, :, :],
                relt[0:64, x * 32:x * 32 + 32],
                qext[0:64, :, x, :],
                start=True, stop=True, tile_position=(0, 64),
            )
            nc.tensor.matmul(
                accr[96:128, :, :],
                relt[0:64, 1024 + x * 32:1024 + x * 32 + 32],
                qext[0:64, :, :, x],
                start=True, stop=True, tile_position=(0, 96),
            )
            if x % 2 == 0:
                nc.scalar.copy(qext[64:96, :, x, :], accr[64:96, :, :])
            else:
                nc.vector.tensor_copy(qext[64:96, :, x, :], accr[64:96, :, :])
            nc.vector.tensor_copy(stg_w[:, x, :, :], accr[96:128, :, :])

        def tpose_w(h):
            # PE transpose: out[j, (a,b)] = stg_w[j, (b,a)] via identity
            # lhsT; the scattered access rides the rhs stream, the PSUM
            # output and its evacuation are contiguous.
            tp = ps_s.tile([32, 2, 512], f32, tag="S", name=f"tp_{h}")
            for hf in range(2):
                nc.tensor.matmul(
                    tp[:, hf, :],
                    iden[:],
                    stg_w[:, :, h, hf * 16:(hf + 1) * 16].rearrange(
                        "p b a -> p a b"),
                    start=True, stop=True,
                )
            nc.vector.tensor_copy(qext[96:128, h, :, :], tp[:])

        for x in range(32):
            rel_iter(x)
            if x % 5 == 4:
                v_tile(x // 5)  # v0..v5 keep the PE busy during rel evacs
        # transposes for the first heads before attention starts
        tpose_w(0)
        tpose_w(1)
        tpose_w(2)

        # ================= attention slots =================
        DEFER = 2
        slots = [(hf, h) for hf in range(2) for h in range(12)]
        p_bufs = {}
        acc_pv = {}

        def emit_S(si):
            hf, h = slots[si]
            p_t = pp.tile([128, 8, 512], bf16, tag="P", name=f"p_{si}")
            p_bufs[si] = p_t
            for jp in range(4):
                accs = ps_s.tile([128, 2, 512], f32, tag="S")
                for half in range(2):
                    kt = 2 * jp + half
                    nc.tensor.matmul(
                        accs[:, half, :],
                        kext[:, h, kt * 4:(kt + 1) * 4, :],
                        qext[:, h, hf * 16:(hf + 1) * 16, :],
                        start=True, stop=True,
                    )
                nc.scalar.activation(p_t[:, 2 * jp:2 * jp + 2, :], accs[:], EXP)

        def emit_PV(si):
            hf, h = slots[si]
            p_t = p_bufs.pop(si)
            accp = ps_pv.tile([65, 512], f32, tag="PV")
            acc_pv[si] = accp
            for kt in range(8):
                nc.tensor.matmul(
                    accp[:],
                    vaug[:, kt, h, :],
                    p_t[:, kt, :],
                    start=(kt == 0), stop=(kt == 7),
                )

        def emit_norm(si):
            hf, h = slots[si]
            accp = acc_pv.pop(si)
            d_sq = np_pool.tile([32, 16], f32, tag="dsq")
            d_row = np_pool.tile([1, 512], f32, tag="drow")
            r_row = np_pool.tile([1, 512], f32, tag="rrow")
            r_bc = np_pool.tile([64, 512], f32, tag="rbc")
            nc.vector.tensor_copy(d_row[:], accp[64:65, :])
            nc.sync.dma_start(d_sq[:], d_row[:])
            nc.vector.reciprocal(d_sq[:], d_sq[:])
            nc.sync.dma_start(r_row[:], d_sq[:])
            nc.gpsimd.partition_broadcast(r_bc[:], r_row[:])
            nc.vector.tensor_mul(
                yall[64 * (h % 2):64 * (h % 2 + 1), h // 2,
                     hf * 512:(hf + 1) * 512],
                accp[0:64, :], r_bc[:])

        def proj_mms(accz, g, p_lo, p_hi, start, stop):
            for p in range(p_lo, p_hi):
                nc.tensor.matmul(
                    accz[:, 0:512],
                    yall[:, p, g * 128:(g + 1) * 128],
                    wpr[:, p, 0:512],
                    start=start and (p == p_lo), stop=stop and (p == p_hi - 1),
                )
                nc.tensor.matmul(
                    accz[:, 512:768],
                    yall[:, p, g * 128:(g + 1) * 128],
                    wpr[:, p, 512:768],
                    start=start and (p == p_lo), stop=stop and (p == p_hi - 1),
                )

        def emit_proj(hf, tt):
            g = hf * 4 + tt
            if hf == 1:
                accz = ps_s.tile([128, C], f32, tag="S", name=f"z_{g}")
            else:
                accz = ps_aux.tile([128, C], f32, tag="aux", name=f"z_{g}")
            proj_mms(accz, g, 0, 6, True, True)
            z_t = zp.tile([128, C], f32, tag="Zt")
            if hf == 1:
                nc.scalar.copy(z_t[:], accz[:])
                nc.vector.tensor_add(z_t[:], z_t[:], bp_bc[:])
            else:
                nc.vector.tensor_add(z_t[:], accz[:], bp_bc[:])
            nc.sync.dma_start(out_d[g * 128:(g + 1) * 128, :], z_t[:])

        # filler schedule: slot -> thunks emitted after that slot's S.
        # Ordering constraints: k o-tile 6+p before S of head 2p (slot 2p);
        # tpose_w(h) before S of head h (slot h); all v tiles before the
        # first PV (slot DEFER).
        fillers = {
            0: [lambda: v_tile(6), lambda: tpose_w(3)],
            1: [lambda: v_tile(7), lambda: tpose_w(4)],
            2: [lambda: qk_otile(ps_aux, 8, "aux"), lambda: tpose_w(5)],
            3: [lambda: qk_otile(ps_aux, 9, "aux"), lambda: tpose_w(6)],
            4: [lambda: qk_otile(ps_aux, 10, "aux"), lambda: tpose_w(7)],
            5: [lambda: qk_otile(ps_aux, 11, "aux"), lambda: tpose_w(8)],
            6: [lambda: tpose_w(9)],
            7: [lambda: tpose_w(10)],
            8: [lambda: tpose_w(11)],
            14: [lambda: emit_proj(0, 0)],
            16: [lambda: emit_proj(0, 1)],
            18: [lambda: emit_proj(0, 2)],
            20: [lambda: emit_proj(0, 3)],
        }

        emit_S(0)
        emit_S(1)
        for f in fillers.get(0, []):
            f()
        for f in fillers.get(1, []):
            f()
        rel_es.close()
        ps_pv = es.enter_context(
            tc.tile_pool(name="ps_pv", bufs=2, space="PSUM"))
        for si in range(2, 24):
            emit_S(si)
            for f in fillers.get(si, []):
                f()
            emit_PV(si - DEFER)
            emit_norm(si - DEFER)
        # tail: PV(22) runs under slot 23's exps; proj tiles 4,5 accumulate
        # progressively on the freed ps_s banks while the last PV/norm
        # chains drain, then the rest finishes with evacs on idle ScalarE.
        emit_PV(22)
        emit_norm(22)
        zacc = {}
        for g in (4, 5):
            zacc[g] = ps_s.tile([128, C], f32, tag="S", name=f"z_{g}")
            proj_mms(zacc[g], g, 0, 5, True, False)
        emit_PV(23)
        emit_norm(23)
        for g in (4, 5):
            proj_mms(zacc[g], g, 5, 6, False, True)
            z_t = zp.tile([128, C], f32, tag="Zt", name=f"zt_{g}")
            nc.scalar.copy(z_t[:], zacc[g][:])
            nc.vector.tensor_add(z_t[:], z_t[:], bp_bc[:])
            nc.sync.dma_start(out_d[g * 128:(g + 1) * 128, :], z_t[:])
        for tt in (2, 3):
            emit_proj(1, tt)

        es.close()

    nc.compile()
    _cache["nc"] = nc
    return nc


def _host_prep(x, w_qkv, b_qkv, w_proj, b_proj, rel_pos_h, rel_pos_w):
    scale = HD ** -0.5
    w_qkv = _f32(w_qkv)
    b_qkv = _f32(b_qkv)

    w_qk = w_qkv[:, : 2 * C].copy()
    w_qk[:, :C] *= scale
    b_qk = b_qkv[:C] * scale  # q bias row (k bias cancels in softmax)

    # relt [64, 2048]: cols tbl*1024 + x*32 + j -> 8*rel_pos[x - j + 31, :]
    idx = np.arange(32)[:, None] - np.arange(32)[None, :] + 31  # [x, j]
    relt = np.concatenate(
        [
            (8.0 * _f32(rel_pos_h))[idx].transpose(2, 0, 1).reshape(64, 1024),
            (8.0 * _f32(rel_pos_w))[idx].transpose(2, 0, 1).reshape(64, 1024),
        ],
        axis=1,
    )

    k = np.arange(T)
    onehot = np.zeros((64, T), np.float32)
    onehot[k // 32, k] = 1.0
    onehot[32 + k % 32, k] = 1.0

    # the value bias passes through softmax unchanged (weights sum to 1),
    # so fold it into the proj bias host-side; the key bias only adds a
    # per-query constant to the logits, which softmax cancels -> dropped.
    b_p_eff = _f32(b_proj) + _f32(b_qkv[2 * C:]) @ _f32(w_proj)
    shared = {
        "w_qk": _bf(w_qk),
        "w_v": _bf(w_qkv[:, 2 * C:]),
        "w_p": _bf(w_proj),
        "b_qk": _bf(b_qk)[None, :],
        "b_p": b_p_eff[None, :],
        "relt": _bf(relt),
        "onehot": _bf(onehot).reshape(64, 32, 32),
        "ident": _bf(np.eye(32)),
    }
    x = _f32(x)
    in_maps = []
    for i in range(N_CORES):
        m = dict(shared)
        m["xT"] = _bf(x[i].reshape(T, C).T)
        in_maps.append(m)
    return in_maps


def kernel(x, w_qkv, b_qkv, w_proj, b_proj, rel_pos_h, rel_pos_w):
    from concourse.bass_utils import run_bass_kernel_spmd

    nc = _build_nc()
    in_maps = _host_prep(x, w_qkv, b_qkv, w_proj, b_proj, rel_pos_h, rel_pos_w)
    res = run_bass_kernel_spmd(nc, in_maps, core_ids=list(range(N_CORES)))
    out = np.stack([_f32(res.results[i]["out"]) for i in range(N_CORES)])
    return out.reshape(B, H, W, C)
